# revision 36
# baseline (speedup 1.0000x reference)
"""NT-Xent loss on 8 Trainium2 NeuronCores (Bass/Tile) — v3.

Math (stacked order rows = [zjs; zis], pair(i) = i +- B):
    cos_ij = (r_i . r_j) * inv_i * inv_j,   inv = 1/|r|
    loss   = 2 + mean_i( ln(rowsum_i - 1) - 2*cos_{i,pair(i)} )
    rowsum_i = sum_j exp(2 cos_ij - 2)   (diagonal contributes exactly 1)

Design (per core, inputs rolled so local rows = cols [0:1024)):
- TRANSPOSED similarity blocks: PSUM block = [128 j-cols, 1024 local-i].
  The j-side norm 1/|r_j| rides the PER-PARTITION scale AP of the exp
  instruction, so only the 2048 local+partner columns are explicitly
  normalized (fp8 znt8); the stationary side stays RAW fp8.
- Main matmul fp8e4 DoubleRow (K=256/instr): stationary = raw fp8
  j-tile, moving = normalized local columns. 2 matmuls per j-tile.
- exp split across THREE pointwise engines (ACT / DVE / GPSIMD):
    ACT: exp8 = fp8(exp((2 inv_j)*S - 2))          [scale AP]
    DVE/GPSIMD: bits = round((23.083 inv_j)*S + 32.517)  [Schraudolph
         e4m3, round-to-nearest uint8 out, C=0.4 calibrated; GPSIMD
         output is bit-identical to DVE]
- Row sums on the PE: ones[128,2,128] fp8-DR matmuls accumulate
  sum_j exp8 into PSUM [128, 512] x2 (broadcast over partitions);
  final = one Ln(bias=-1)+accum per half.
- Norms: squares are written as fp8e4 in k-interleaved layout
  [128, KC, CHUNK], so one fp8-DoubleRow ones-matmul per 512-quarter
  computes the colsum (K=256, 0.5 cyc/col): 4x cheaper than the bf16
  row-layout colsums. c0/c2 keep the row path (Ln + Exp on [1,512]
  rows) to produce inv rows for the normalize broadcasts; c1/c3 skip
  row-Lns entirely: raw ss rows bounce PSUM->DRAM->[128,16] compact
  (DRAM APs transpose freely) and Ln+Exp run on the tiny compact tile.
"""

import numpy as np
from contextlib import ExitStack

import concourse.bass as bass
import concourse.tile as tile
from concourse import bacc, mybir
from concourse.bass_utils import run_bass_kernel_spmd
from concourse._compat import with_exitstack

B = 4096
D = 256
N = 2 * B                 # 8192 similarity rows/cols
N_CORES = 8
LOCAL = N // N_CORES      # 1024 rows per core
CHUNK = 2048
NCHUNK = N // CHUNK       # 4
KC = D // 128             # 2 contraction subtiles of 128
NTILES = N // 128         # 64 j-tiles
TPC = CHUNK // 128        # 16 j-tiles per chunk

F32 = mybir.dt.float32
F32R = mybir.dt.float32r
BF16 = mybir.dt.bfloat16
FP8 = mybir.dt.float8e4
U8 = mybir.dt.uint8
AF = mybir.ActivationFunctionType
ALU = mybir.AluOpType
X = mybir.AxisListType.X
DR = mybir.MatmulPerfMode.DoubleRow

LN2 = float(np.log(2.0))
A8 = 8.0 / LN2                      # e4m3 Schraudolph slope per unit arg
SCHR_B = 56.0 - 2.0 * A8 - 0.4     # bits = (2*A8*inv)*S + SCHR_B
LN_2A8 = float(np.log(2.0 * A8))   # ACT bias producing (2*A8)*inv
SCHR_A = 2.0 * A8                  # const slope for normalized blocks
SUM_LAG = 5                        # sum-matmul software pipelining lag

CORDER = (0, 2, 1, 3)              # local, partner, rest

# per-chunk (ACT, DVE) tile quotas; pairs interleave par0->ACT par1->DVE
ENGINE_QUOTA = {0: 8, 2: 8, 1: 9, 3: 8}   # ACT tiles per chunk


def _engine_map():
    m = {}
    for c in range(4):
        a_left = ENGINE_QUOTA[c]
        d_left = 16 - a_left
        for q in range(8):
            for par in range(2):
                tt = 2 * q + par
                want_act = (par == 0)
                if want_act and a_left > 0 or d_left == 0:
                    m[(c, tt)] = "act"; a_left -= 1
                else:
                    m[(c, tt)] = "dve"; d_left -= 1
    return m


ENGINE_MAP = _engine_map()


@with_exitstack
def _ntxent_kernel(ctx: ExitStack, tc: tile.TileContext, rt_ap, out_ap):
    nc = tc.nc

    sb_rt = ctx.enter_context(tc.tile_pool(name="rt", bufs=2 * NCHUNK))
    sb_big = ctx.enter_context(tc.tile_pool(name="big", bufs=1))
    sb_sq = ctx.enter_context(tc.tile_pool(name="sq", bufs=4))
    sb_e8 = ctx.enter_context(tc.tile_pool(name="e8", bufs=8))
    sb_fin = ctx.enter_context(tc.tile_pool(name="fin", bufs=1))
    dram = ctx.enter_context(tc.tile_pool(name="dram", bufs=1, space="DRAM"))

    # ---- constants ----
    ones_r = sb_fin.tile([1, 128], BF16, tag="ones_r")      # K=1 bcast
    nc.vector.memset(ones_r[:], 1.0)
    ones8 = sb_fin.tile([128, 2, 128], FP8, tag="ones8")    # DR row-sum
    nc.vector.memset(ones8[:], 1.0)
    neg1 = sb_fin.tile([128, 1], F32, tag="neg1")
    nc.vector.memset(neg1[:], -1.0)
    neg2 = sb_fin.tile([128, 1], F32, tag="neg2")
    nc.vector.memset(neg2[:], -2.0)
    bln2 = sb_fin.tile([128, 1], F32, tag="bln2")
    nc.vector.memset(bln2[:], LN2)
    blnA = sb_fin.tile([128, 1], F32, tag="blnA")
    nc.vector.memset(blnA[:], LN_2A8)

    # ---- persistent tiles ----
    rt8 = sb_big.tile([128, KC, N], FP8, tag="rt8")            # raw fp8
    znt8 = sb_fin.tile([128, KC, 2 * CHUNK], FP8, tag="znt8")  # normed c0|c2
    pacc = sb_fin.tile([128, KC], F32, tag="pacc")
    fin2 = sb_fin.tile([128, 2], F32, tag="fin2")

    lnrow = {}      # per chunk: ln(ss) row layout [1, 2048] f32
    lncomp = {}     # c1/c3: ln(ss) compact [128, 16] (DMA-transposed)
    inv2c = {}      # scale 2*inv
    invAc = {}      # scale 23.083*inv
    for c in range(NCHUNK):
        lnrow[c] = sb_fin.tile([1, CHUNK], F32, tag=f"lnrow{c}", name=f"lnrow{c}")
    for c in (1, 3):
        lncomp[c] = sb_fin.tile([128, TPC], F32, tag=f"lncomp{c}", name=f"lncomp{c}")
        inv2c[c] = sb_fin.tile([128, TPC], F32, tag=f"inv2c{c}", name=f"inv2c{c}")
        invAc[c] = sb_fin.tile([128, TPC], F32, tag=f"invAc{c}", name=f"invAc{c}")
    # plain-inv rows (bf16) for chunks 0/2
    invrow = {}
    for nm in ("l", "p"):
        invrow[nm] = sb_fin.tile([1, CHUNK], BF16, tag=f"invrow{nm}", name=f"invrow{nm}")

    dmy_pos = [sb_fin.tile([128, LOCAL], BF16, tag=f"dmy_pos{k}",
                           name=f"dmy_pos{k}") for k in range(KC)]
    dmy_ln = sb_fin.tile([128, 512], BF16, tag="dmy_ln")

    # ---- input DMAs ----
    # c0/c2 arrive as 512-col quarters ordered quarter-major so the
    # per-quarter norm chains can chase the data; c1/c3 as whole chunks.
    rtk = {}
    for c in CORDER:
        for k in range(KC):
            rtk[(c, k)] = sb_rt.tile([128, CHUNK], BF16, tag="rt",
                                     name=f"rt_{c}_{k}")
    for c in CORDER:
        if c in (0, 2):
            for h in range(4):
                for k in range(KC):
                    hs = bass.ds(h * 512, 512)
                    nc.sync.dma_start(
                        out=rtk[(c, k)][0:128, hs],
                        in_=rt_ap[k][:, bass.ds(c * CHUNK + h * 512, 512)])
        else:
            for k in range(KC):
                nc.sync.dma_start(out=rtk[(c, k)][:],
                                  in_=rt_ap[k][:, bass.ds(c * CHUNK, CHUNK)])

    # squares, fp8 k-interleaved: sqt[c] = [128, KC, CHUNK]
    sqt = {}
    for c in range(NCHUNK):
        sqt[c] = sb_sq.tile([128, KC, CHUNK], FP8, tag="sq", name=f"sq_{c}")

    def squares(c, engine=None):
        eng = engine or nc.gpsimd
        for k in range(KC):
            eng.tensor_tensor(sqt[c][:, k, :], rtk[(c, k)][:], rtk[(c, k)][:],
                              ALU.mult)

    def cast8(c, eng="dve"):
        for k in range(KC):
            if eng == "dve":
                nc.vector.tensor_copy(rt8[:, k, bass.ds(c * CHUNK, CHUNK)],
                                      rtk[(c, k)][:])
            elif eng == "gp":
                # mult-by-1 tensor_scalar: ~2x faster than gpsimd copy
                nc.gpsimd.tensor_scalar(rt8[:, k, bass.ds(c * CHUNK, CHUNK)],
                                        rtk[(c, k)][:], 1.0, 0.0,
                                        ALU.mult, ALU.add)
            else:
                nc.scalar.activation(rt8[:, k, bass.ds(c * CHUNK, CHUNK)],
                                     rtk[(c, k)][:], AF.Copy,
                                     bias=0.0, scale=1.0)

    def squares_q(c, qq, eng):
        qsl = bass.ds(qq * 512, 512)
        for k in range(KC):
            if eng == "dve":
                nc.vector.tensor_mul(sqt[c][:, k, qsl], rtk[(c, k)][:, qsl],
                                     rtk[(c, k)][:, qsl])
            else:
                nc.gpsimd.tensor_tensor(sqt[c][:, k, qsl],
                                        rtk[(c, k)][:, qsl],
                                        rtk[(c, k)][:, qsl], ALU.mult)

    # bulk emission keeps every square op unblocked in its engine's
    # lookahead window while the chain muls wait on the bcast roundtrip
    for qq in range(4):
        squares_q(0, qq, "dve")
    for qq in range(4):
        squares_q(2, qq, "gp")

    # standalone weight loads: free PE activity to hold the p-state up
    # through windows where the matmul stream has a dependency bubble
    def pe_fill(n):
        for _ in range(n):
            nc.tensor.ldweights(ones8[:], perf_mode=DR)

    # =====================  chunk-0 norm (scoped PSUM)  =================
    # chunk 0: per-quarter chain rowss -> Ln -> Exp -> bcast -> apply
    def c0_quarter(qq, rpool, bpool):
        off = qq * 512
        osl = bass.ds(off, 512)
        rb = rpool.tile([128, 512], F32, tag="rowss", name=f"rq_{qq}")
        nc.tensor.matmul(rb[:], ones8[:], sqt[0][:, :, osl],
                         start=True, stop=True, perf_mode=DR)
        nc.scalar.activation(lnrow[0][0:1, osl], rb[0:1, :],
                             AF.Ln, bias=0.0, scale=1.0)
        nc.scalar.activation(invrow["l"][0:1, osl], lnrow[0][0:1, osl],
                             AF.Exp, bias=0.0, scale=-0.5)
        bcb = bpool.tile([128, 512], F32, tag="invb", name=f"invb_{qq}")
        nc.tensor.matmul(bcb[:], ones_r[:], invrow["l"][0:1, osl],
                         start=True, stop=True)
        for k in range(KC):
            nc.vector.tensor_mul(znt8[:, k, osl],
                                 rtk[(0, k)][:, osl], bcb[:])

    with ExitStack() as norm_ctx:
        ps_row = norm_ctx.enter_context(
            tc.tile_pool(name="psrow", bufs=2, space="PSUM"))
        ps_bc = norm_ctx.enter_context(
            tc.tile_pool(name="psbc", bufs=2, space="PSUM"))
        ps_warm = norm_ctx.enter_context(
            tc.tile_pool(name="pswarm", bufs=1, space="PSUM"))

        # PE p-state warm-up while waiting for the first colsum input.
        warm = ps_warm.tile([128, 128], F32, tag="warm")
        for _ in range(8):
            nc.tensor.matmul(warm[:], ones8[:], ones8[:],
                             start=True, stop=True, perf_mode=DR,
                             skip_group_check=True)
        for qq in range(4):
            c0_quarter(qq, ps_row, ps_bc)

    # =====================  main phase  =================================
    ps_mm = ctx.enter_context(tc.tile_pool(name="psmm", bufs=3, space="PSUM"))
    ps_sum = ctx.enter_context(tc.tile_pool(name="pssum", bufs=1, space="PSUM"))

    sums = [ps_sum.tile([128, 512], F32, tag=f"sum{h}", name=f"sum{h}")
            for h in range(2)]

    n_pairs = NTILES // 2
    pair_list = []
    for c in CORDER:
        for q in range(TPC // 2):
            pair_list.append((c, q))

    # stationary for j-tile t: normalized znt8 slice for chunks 0/2
    # (znt8 = [local | partner]), raw rt8 for chunks 1/3
    def stationary(c, tt):
        if c == 0:
            return znt8[:, :, bass.ds(tt * 128, 128)]
        if c == 2:
            return znt8[:, :, bass.ds(CHUNK + tt * 128, 128)]
        return rt8[:, :, bass.ds((c * TPC + tt) * 128, 128)]

    e8s = {}

    def emit_sum(pj, first, last):
        e8p = e8s.pop(pj)
        for h in range(2):
            nc.tensor.matmul(sums[h][:], ones8[:],
                             e8p[:, :, bass.ds(h * 512, 512)],
                             start=first, stop=last,
                             perf_mode=DR, skip_group_check=True)

    # c1/c3 norm quarter: one fp8-DR colsum matmul + Ln of the
    # (partition-broadcast) PSUM row into the SBUF ln row
    def ln_quarter(c, qq):
        rb = ps_mm.tile([128, LOCAL], F32, tag="mm", name=f"rl_{c}_{qq}")
        sl = bass.ds(qq * 512, 512)
        nc.tensor.matmul(rb[:, 0:512], ones8[:], sqt[c][:, :, sl],
                         start=True, stop=True, perf_mode=DR)
        nc.scalar.activation(lnrow[c][0:1, sl], rb[0:1, 0:512],
                             AF.Ln, bias=0.0, scale=1.0)

    # c1/c3: ln rows -> compact [128, 16] via a DRAM bounce (DRAM APs
    # transpose freely), then the two scale tiles
    def ln_compact(c):
        lnd = dram.tile([1, CHUNK], F32, tag=f"lnd{c}", name=f"lnd{c}")
        nc.sync.dma_start(out=lnd[:, :], in_=lnrow[c][:, :])
        nc.sync.dma_start(
            out=lncomp[c][:],
            in_=lnd[0:1, :].rearrange("o (t p) -> (o p) t", p=128))
        nc.scalar.activation(inv2c[c][:], lncomp[c][:], AF.Exp,
                             bias=bln2[:], scale=-0.5)
        nc.scalar.activation(invAc[c][:], lncomp[c][:], AF.Exp,
                             bias=blnA[:], scale=-0.5)

    # chunk-2 norm, injected into the loop. Quarter-chained exactly like
    # the chunk-0 head chain so partner znt8 becomes usable per-512-slab:
    # sq q -> rowss q -> Ln q -> Exp q -> bcast q -> muls q
    def c2_norm_quarter(qq):
        off = qq * 512
        osl = bass.ds(off, 512)
        rb = ps_mm.tile([128, LOCAL], F32, tag="mm", name=f"rl_2_{qq}")
        nc.tensor.matmul(rb[:, 0:512], ones8[:], sqt[2][:, :, osl],
                         start=True, stop=True, perf_mode=DR)
        nc.scalar.activation(lnrow[2][0:1, osl], rb[0:1, 0:512],
                             AF.Ln, bias=0.0, scale=1.0)
        nc.scalar.activation(invrow["p"][0:1, osl], lnrow[2][0:1, osl],
                             AF.Exp, bias=0.0, scale=-0.5)
        bcb = ps_mm.tile([128, LOCAL], F32, tag="mm", name=f"invbl_{qq}")
        bc = bcb[:, 0:512]
        nc.tensor.matmul(bc, ones_r[:], invrow["p"][0:1, osl],
                         start=True, stop=True)
        for k in range(KC):
            nc.vector.tensor_mul(znt8[:, k, bass.ds(CHUNK + off, 512)],
                                 rtk[(2, k)][:, osl], bc)

    pair_counter = [0]

    def emit_pair(c, q):
        pi = pair_counter[0]
        pair_counter[0] += 1
        e8 = sb_e8.tile([128, 2, LOCAL], FP8, tag="e8")
        e8s[pi] = e8
        for par in range(2):
            tt = 2 * q + par
            pm = ps_mm.tile([128, LOCAL], F32, tag="mm")
            for h in range(2):
                nc.tensor.matmul(pm[:, bass.ds(h * 512, 512)],
                                 stationary(c, tt),
                                 znt8[:, :, bass.ds(h * 512, 512)],
                                 start=True, stop=True, perf_mode=DR)
            normed = c in (0, 2)
            eng = ENGINE_MAP[(c, tt)]
            if eng == "act":
                if normed:
                    nc.scalar.activation(e8[:, par, :], pm[:], AF.Exp,
                                         bias=neg2[:], scale=2.0)
                else:
                    nc.scalar.activation(e8[:, par, :], pm[:], AF.Exp,
                                         bias=neg2[:],
                                         scale=inv2c[c][:, bass.ds(tt, 1)])
            else:
                if normed:
                    nc.vector.tensor_scalar(e8[:, par, :].bitcast(U8), pm[:],
                                            SCHR_A, SCHR_B,
                                            ALU.mult, ALU.add)
                else:
                    nc.vector.tensor_scalar(e8[:, par, :].bitcast(U8), pm[:],
                                            invAc[c][:, bass.ds(tt, 1)],
                                            SCHR_B, ALU.mult, ALU.add)
        if pi >= SUM_LAG:
            emit_sum(pi - SUM_LAG, pi == SUM_LAG, False)

    def emit_pos():
        # positive-pair cosines (fp8 TT on GPSIMD + free-dim reduce on
        # DVE); pool ops land after the prep queue, reduces ride the
        # DVE stream
        for k in range(KC):
            nc.gpsimd.tensor_tensor(dmy_pos[k][:], znt8[:, k, 0:LOCAL],
                                    znt8[:, k, bass.ds(CHUNK, LOCAL)],
                                    ALU.mult)
        for k in range(KC):
            nc.vector.reduce_sum(pacc[:, bass.ds(k, 1)], dmy_pos[k][:],
                                 axis=X)

    # ---- emission: c0 pairs first; the c2 quarter chains ride along,
    # then c2 pairs pad the stretch where c1's norm and casts run ----
    for q in (0, 1, 2):
        emit_pair(0, q)
    c2_norm_quarter(0)
    emit_pair(0, 3)
    c2_norm_quarter(1)
    emit_pair(0, 4)
    c2_norm_quarter(2)
    emit_pair(0, 5)
    c2_norm_quarter(3)
    for q in (6, 7):
        emit_pair(0, q)
    # pool prep for the raw chunks (gpsimd order: after the c2 squares)
    squares(1)
    cast8(1, "gp")
    squares(3)
    cast8(3, "gp")
    for q in (0, 1, 2, 3):
        emit_pair(2, q)
    ln_quarter(1, 0)
    emit_pair(2, 4)
    ln_quarter(1, 1)
    emit_pair(2, 5)
    ln_quarter(1, 2)
    emit_pair(2, 6)
    ln_quarter(1, 3)
    emit_pair(2, 7)
    ln_compact(1)
    for q in (0, 1, 2, 3):
        emit_pair(1, q)
    ln_quarter(3, 0)
    emit_pair(1, 4)
    ln_quarter(3, 1)
    emit_pair(1, 5)
    ln_quarter(3, 2)
    emit_pair(1, 6)
    ln_quarter(3, 3)
    emit_pair(1, 7)
    ln_compact(3)
    emit_pos()
    for q in range(8):
        emit_pair(3, q)
    for pj in range(n_pairs - SUM_LAG, n_pairs):
        emit_sum(pj, False, pj == n_pairs - 1)

    # =====================  final reduction  ============================
    for h in range(2):
        nc.scalar.activation(dmy_ln[:], sums[h][:], AF.Ln, bias=neg1[:],
                             scale=1.0, accum_out=fin2[:, bass.ds(h, 1)])
    nc.sync.dma_start(out=out_ap[:, 0:2], in_=fin2[:])
    nc.sync.dma_start(out=out_ap[:, 2:4], in_=pacc[:])


def _pin_act_table(nc):
    # natural_log_exp_and_others (id 6) holds Ln, Exp, Copy: zero reloads.
    nc.scalar.add_instruction(mybir.InstLoadActFuncSet(
        name=nc.get_next_instruction_name(), ins=[], outs=[],
        act_func_set_id=6))


_NC_CACHE = None


def _build_program():
    global _NC_CACHE
    if _NC_CACHE is not None:
        return _NC_CACHE
    nc = bacc.Bacc("TRN2", target_bir_lowering=False, debug=False,
                   num_devices=N_CORES)
    rt = nc.dram_tensor("rt", [KC, 128, N], BF16, kind="ExternalInput").ap()
    out = nc.dram_tensor("out", [128, 4], F32, kind="ExternalOutput").ap()
    with tile.TileContext(nc) as tc:
        _pin_act_table(nc)
        _ntxent_kernel(tc, rt, out)
    nc.finalize()
    _NC_CACHE = nc
    return nc


def _prep_inputs(zis: np.ndarray, zjs: np.ndarray) -> list[dict]:
    # Host prep (layout-level only): stack, transpose to [D, N], split
    # the contraction dim, narrow to bf16, and roll columns so each
    # core's local block is at a uniform offset.
    import ml_dtypes
    rt_full = np.ascontiguousarray(
        np.concatenate([zjs, zis], axis=0).T.astype(ml_dtypes.bfloat16)
    ).reshape(KC, 128, N)

    in_maps = []
    for c in range(N_CORES):
        rolled = np.roll(rt_full, -c * LOCAL, axis=2)
        in_maps.append({"rt": np.ascontiguousarray(rolled)})
    return in_maps


def kernel(zis: np.ndarray, zjs: np.ndarray) -> np.ndarray:
    assert zis.shape == (B, D) and zjs.shape == (B, D)
    nc = _build_program()
    in_maps = _prep_inputs(zis, zjs)
    res = run_bass_kernel_spmd(nc, in_maps, core_ids=list(range(N_CORES)))

    log_sum = 0.0
    pos_sum = 0.0
    for c in range(N_CORES):
        o = res.results[c]["out"]
        log_sum += float(o[0, 0]) + float(o[0, 1])
        pos_sum += float(o[:, 2].sum()) + float(o[:, 3].sum())
    loss = 2.0 + (log_sum - 2.0 * pos_sum) / N
    return np.asarray(loss, dtype=np.float32)


# revision 37
# speedup vs baseline: 1.0494x; 1.0494x over previous
"""NT-Xent loss on 8 Trainium2 NeuronCores (Bass/Tile) — v3.

Math (stacked order rows = [zjs; zis], pair(i) = i +- B):
    cos_ij = (r_i . r_j) * inv_i * inv_j,   inv = 1/|r|
    loss   = 2 + mean_i( ln(rowsum_i - 1) - 2*cos_{i,pair(i)} )
    rowsum_i = sum_j exp(2 cos_ij - 2)   (diagonal contributes exactly 1)

Design (per core, inputs rolled so local rows = cols [0:1024)):
- TRANSPOSED similarity blocks: PSUM block = [128 j-cols, 1024 local-i].
  The j-side norm 1/|r_j| rides the PER-PARTITION scale AP of the exp
  instruction, so only the 2048 local+partner columns are explicitly
  normalized (fp8 znt8); the stationary side stays RAW fp8.
- Main matmul fp8e4 DoubleRow (K=256/instr): stationary = raw fp8
  j-tile, moving = normalized local columns. 2 matmuls per j-tile.
- exp split across THREE pointwise engines (ACT / DVE / GPSIMD):
    ACT: exp8 = fp8(exp((2 inv_j)*S - 2))          [scale AP]
    DVE/GPSIMD: bits = round((23.083 inv_j)*S + 32.517)  [Schraudolph
         e4m3, round-to-nearest uint8 out, C=0.4 calibrated; GPSIMD
         output is bit-identical to DVE]
- Row sums on the PE: ones[128,2,128] fp8-DR matmuls accumulate
  sum_j exp8 into PSUM [128, 512] x2 (broadcast over partitions);
  final = one Ln(bias=-1)+accum per half.
- Norms: squares are written as fp8e4 in k-interleaved layout
  [128, KC, CHUNK], so one fp8-DoubleRow ones-matmul per 512-quarter
  computes the colsum (K=256, 0.5 cyc/col): 4x cheaper than the bf16
  row-layout colsums. c0/c2 keep the row path (Ln + Exp on [1,512]
  rows) to produce inv rows for the normalize broadcasts; c1/c3 skip
  row-Lns entirely: raw ss rows bounce PSUM->DRAM->[128,16] compact
  (DRAM APs transpose freely) and Ln+Exp run on the tiny compact tile.
"""

import numpy as np
from contextlib import ExitStack

import concourse.bass as bass
import concourse.tile as tile
from concourse import bacc, mybir
from concourse.bass_utils import run_bass_kernel_spmd
from concourse._compat import with_exitstack

B = 4096
D = 256
N = 2 * B                 # 8192 similarity rows/cols
N_CORES = 8
LOCAL = N // N_CORES      # 1024 rows per core
CHUNK = 2048
NCHUNK = N // CHUNK       # 4
KC = D // 128             # 2 contraction subtiles of 128
NTILES = N // 128         # 64 j-tiles
TPC = CHUNK // 128        # 16 j-tiles per chunk

F32 = mybir.dt.float32
F32R = mybir.dt.float32r
BF16 = mybir.dt.bfloat16
FP8 = mybir.dt.float8e4
U8 = mybir.dt.uint8
AF = mybir.ActivationFunctionType
ALU = mybir.AluOpType
X = mybir.AxisListType.X
DR = mybir.MatmulPerfMode.DoubleRow

LN2 = float(np.log(2.0))
A8 = 8.0 / LN2                      # e4m3 Schraudolph slope per unit arg
SCHR_B = 56.0 - 2.0 * A8 - 0.4     # bits = (2*A8*inv)*S + SCHR_B
LN_2A8 = float(np.log(2.0 * A8))   # ACT bias producing (2*A8)*inv
SCHR_A = 2.0 * A8                  # const slope for normalized blocks
SUM_LAG = 5                        # sum-matmul software pipelining lag

CORDER = (0, 2, 1, 3)              # local, partner, rest

# per-chunk (ACT, DVE) tile quotas; pairs interleave par0->ACT par1->DVE
ENGINE_QUOTA = {0: 8, 2: 8, 1: 9, 3: 8}   # ACT tiles per chunk


def _engine_map():
    m = {}
    for c in range(4):
        a_left = ENGINE_QUOTA[c]
        d_left = 16 - a_left
        for q in range(8):
            for par in range(2):
                tt = 2 * q + par
                want_act = (par == 0)
                if want_act and a_left > 0 or d_left == 0:
                    m[(c, tt)] = "act"; a_left -= 1
                else:
                    m[(c, tt)] = "dve"; d_left -= 1
    return m


ENGINE_MAP = _engine_map()


@with_exitstack
def _ntxent_kernel(ctx: ExitStack, tc: tile.TileContext, rt_ap, out_ap):
    nc = tc.nc

    sb_rt = ctx.enter_context(tc.tile_pool(name="rt", bufs=2 * NCHUNK))
    sb_big = ctx.enter_context(tc.tile_pool(name="big", bufs=1))
    sb_sq = ctx.enter_context(tc.tile_pool(name="sq", bufs=4))
    sb_e8 = ctx.enter_context(tc.tile_pool(name="e8", bufs=8))
    sb_fin = ctx.enter_context(tc.tile_pool(name="fin", bufs=1))
    dram = ctx.enter_context(tc.tile_pool(name="dram", bufs=1, space="DRAM"))

    # ---- constants ----
    ones_r = sb_fin.tile([1, 128], BF16, tag="ones_r")      # K=1 bcast
    nc.vector.memset(ones_r[:], 1.0)
    ones8 = sb_fin.tile([128, 2, 128], FP8, tag="ones8")    # DR row-sum
    nc.vector.memset(ones8[:], 1.0)
    neg1 = sb_fin.tile([128, 1], F32, tag="neg1")
    nc.vector.memset(neg1[:], -1.0)
    neg2 = sb_fin.tile([128, 1], F32, tag="neg2")
    nc.vector.memset(neg2[:], -2.0)
    bln2 = sb_fin.tile([128, 1], F32, tag="bln2")
    nc.vector.memset(bln2[:], LN2)
    blnA = sb_fin.tile([128, 1], F32, tag="blnA")
    nc.vector.memset(blnA[:], LN_2A8)

    # ---- persistent tiles ----
    rt8 = sb_big.tile([128, KC, N], FP8, tag="rt8")            # raw fp8
    znt8 = sb_fin.tile([128, KC, 2 * CHUNK], FP8, tag="znt8")  # normed c0|c2
    pacc = sb_fin.tile([128, KC], F32, tag="pacc")
    fin2 = sb_fin.tile([128, 2], F32, tag="fin2")

    lnrow = {}      # per chunk: ln(ss) row layout [1, 2048] f32
    lncomp = {}     # c1/c3: ln(ss) compact [128, 16] (DMA-transposed)
    inv2c = {}      # scale 2*inv
    invAc = {}      # scale 23.083*inv
    for c in range(NCHUNK):
        lnrow[c] = sb_fin.tile([1, CHUNK], F32, tag=f"lnrow{c}", name=f"lnrow{c}")
    for c in (1, 3):
        lncomp[c] = sb_fin.tile([128, TPC], F32, tag=f"lncomp{c}", name=f"lncomp{c}")
        inv2c[c] = sb_fin.tile([128, TPC], F32, tag=f"inv2c{c}", name=f"inv2c{c}")
        invAc[c] = sb_fin.tile([128, TPC], F32, tag=f"invAc{c}", name=f"invAc{c}")
    # plain-inv rows (bf16) for chunks 0/2
    invrow = {}
    for nm in ("l", "p"):
        invrow[nm] = sb_fin.tile([1, CHUNK], BF16, tag=f"invrow{nm}", name=f"invrow{nm}")

    dmy_pos = [sb_fin.tile([128, LOCAL], BF16, tag=f"dmy_pos{k}",
                           name=f"dmy_pos{k}") for k in range(KC)]
    dmy_ln = sb_fin.tile([128, 512], BF16, tag="dmy_ln")

    # ---- input DMAs ----
    # c0/c2 arrive as 512-col quarters ordered quarter-major so the
    # per-quarter norm chains can chase the data; c1/c3 as whole chunks.
    rtk = {}
    for c in CORDER:
        for k in range(KC):
            rtk[(c, k)] = sb_rt.tile([128, CHUNK], F32, tag="rt",
                                     name=f"rt_{c}_{k}")
    for c in CORDER:
        if c in (0, 2):
            for h in range(4):
                for k in range(KC):
                    hs = bass.ds(h * 512, 512)
                    nc.sync.dma_start(
                        out=rtk[(c, k)][0:128, hs],
                        in_=rt_ap[k][:, bass.ds(c * CHUNK + h * 512, 512)])
        else:
            for k in range(KC):
                nc.sync.dma_start(out=rtk[(c, k)][:],
                                  in_=rt_ap[k][:, bass.ds(c * CHUNK, CHUNK)])

    # squares, fp8 k-interleaved: sqt[c] = [128, KC, CHUNK]
    sqt = {}
    for c in range(NCHUNK):
        sqt[c] = sb_sq.tile([128, KC, CHUNK], FP8, tag="sq", name=f"sq_{c}")

    def squares(c, engine=None):
        eng = engine or nc.gpsimd
        for k in range(KC):
            eng.tensor_tensor(sqt[c][:, k, :], rtk[(c, k)][:], rtk[(c, k)][:],
                              ALU.mult)

    def cast8(c, eng="dve"):
        for k in range(KC):
            if eng == "dve":
                nc.vector.tensor_copy(rt8[:, k, bass.ds(c * CHUNK, CHUNK)],
                                      rtk[(c, k)][:])
            elif eng == "gp":
                # mult-by-1 tensor_scalar: ~2x faster than gpsimd copy
                nc.gpsimd.tensor_scalar(rt8[:, k, bass.ds(c * CHUNK, CHUNK)],
                                        rtk[(c, k)][:], 1.0, 0.0,
                                        ALU.mult, ALU.add)
            else:
                nc.scalar.activation(rt8[:, k, bass.ds(c * CHUNK, CHUNK)],
                                     rtk[(c, k)][:], AF.Copy,
                                     bias=0.0, scale=1.0)

    def squares_q(c, qq, eng):
        qsl = bass.ds(qq * 512, 512)
        for k in range(KC):
            if eng == "dve":
                nc.vector.tensor_mul(sqt[c][:, k, qsl], rtk[(c, k)][:, qsl],
                                     rtk[(c, k)][:, qsl])
            else:
                nc.gpsimd.tensor_tensor(sqt[c][:, k, qsl],
                                        rtk[(c, k)][:, qsl],
                                        rtk[(c, k)][:, qsl], ALU.mult)

    # bulk emission keeps every square op unblocked in its engine's
    # lookahead window while the chain muls wait on the bcast roundtrip
    for qq in range(4):
        squares_q(0, qq, "dve")
    for qq in range(4):
        squares_q(2, qq, "gp")

    # standalone weight loads: free PE activity to hold the p-state up
    # through windows where the matmul stream has a dependency bubble
    def pe_fill(n):
        for _ in range(n):
            nc.tensor.ldweights(ones8[:], perf_mode=DR)

    # =====================  chunk-0 norm (scoped PSUM)  =================
    # chunk 0: per-quarter chain rowss -> Ln -> Exp -> bcast -> apply
    def c0_quarter(qq, rpool, bpool):
        off = qq * 512
        osl = bass.ds(off, 512)
        rb = rpool.tile([128, 512], F32, tag="rowss", name=f"rq_{qq}")
        nc.tensor.matmul(rb[:], ones8[:], sqt[0][:, :, osl],
                         start=True, stop=True, perf_mode=DR)
        nc.scalar.activation(lnrow[0][0:1, osl], rb[0:1, :],
                             AF.Ln, bias=0.0, scale=1.0)
        nc.scalar.activation(invrow["l"][0:1, osl], lnrow[0][0:1, osl],
                             AF.Exp, bias=0.0, scale=-0.5)
        bcb = bpool.tile([128, 512], F32, tag="invb", name=f"invb_{qq}")
        nc.tensor.matmul(bcb[:], ones_r[:], invrow["l"][0:1, osl],
                         start=True, stop=True)
        for k in range(KC):
            nc.vector.tensor_mul(znt8[:, k, osl],
                                 rtk[(0, k)][:, osl], bcb[:])

    with ExitStack() as norm_ctx:
        ps_row = norm_ctx.enter_context(
            tc.tile_pool(name="psrow", bufs=2, space="PSUM"))
        ps_bc = norm_ctx.enter_context(
            tc.tile_pool(name="psbc", bufs=2, space="PSUM"))
        ps_warm = norm_ctx.enter_context(
            tc.tile_pool(name="pswarm", bufs=1, space="PSUM"))

        # PE p-state warm-up while waiting for the first colsum input.
        warm = ps_warm.tile([128, 128], F32, tag="warm")
        for _ in range(8):
            nc.tensor.matmul(warm[:], ones8[:], ones8[:],
                             start=True, stop=True, perf_mode=DR,
                             skip_group_check=True)
        for qq in range(4):
            c0_quarter(qq, ps_row, ps_bc)

    # =====================  main phase  =================================
    ps_mm = ctx.enter_context(tc.tile_pool(name="psmm", bufs=3, space="PSUM"))
    ps_sum = ctx.enter_context(tc.tile_pool(name="pssum", bufs=1, space="PSUM"))

    sums = [ps_sum.tile([128, 512], F32, tag=f"sum{h}", name=f"sum{h}")
            for h in range(2)]

    n_pairs = NTILES // 2
    pair_list = []
    for c in CORDER:
        for q in range(TPC // 2):
            pair_list.append((c, q))

    # stationary for j-tile t: normalized znt8 slice for chunks 0/2
    # (znt8 = [local | partner]), raw rt8 for chunks 1/3
    def stationary(c, tt):
        if c == 0:
            return znt8[:, :, bass.ds(tt * 128, 128)]
        if c == 2:
            return znt8[:, :, bass.ds(CHUNK + tt * 128, 128)]
        return rt8[:, :, bass.ds((c * TPC + tt) * 128, 128)]

    e8s = {}

    def emit_sum(pj, first, last):
        e8p = e8s.pop(pj)
        for h in range(2):
            nc.tensor.matmul(sums[h][:], ones8[:],
                             e8p[:, :, bass.ds(h * 512, 512)],
                             start=first, stop=last,
                             perf_mode=DR, skip_group_check=True)

    # c1/c3 norm quarter: one fp8-DR colsum matmul + Ln of the
    # (partition-broadcast) PSUM row into the SBUF ln row
    def ln_quarter(c, qq):
        rb = ps_mm.tile([128, LOCAL], F32, tag="mm", name=f"rl_{c}_{qq}")
        sl = bass.ds(qq * 512, 512)
        nc.tensor.matmul(rb[:, 0:512], ones8[:], sqt[c][:, :, sl],
                         start=True, stop=True, perf_mode=DR)
        nc.scalar.activation(lnrow[c][0:1, sl], rb[0:1, 0:512],
                             AF.Ln, bias=0.0, scale=1.0)

    # c1/c3: ln rows -> compact [128, 16] via a DRAM bounce (DRAM APs
    # transpose freely), then the two scale tiles
    def ln_compact(c):
        lnd = dram.tile([1, CHUNK], F32, tag=f"lnd{c}", name=f"lnd{c}")
        nc.sync.dma_start(out=lnd[:, :], in_=lnrow[c][:, :])
        nc.sync.dma_start(
            out=lncomp[c][:],
            in_=lnd[0:1, :].rearrange("o (t p) -> (o p) t", p=128))
        nc.scalar.activation(inv2c[c][:], lncomp[c][:], AF.Exp,
                             bias=bln2[:], scale=-0.5)
        nc.scalar.activation(invAc[c][:], lncomp[c][:], AF.Exp,
                             bias=blnA[:], scale=-0.5)

    # chunk-2 norm, injected into the loop. Quarter-chained exactly like
    # the chunk-0 head chain so partner znt8 becomes usable per-512-slab:
    # sq q -> rowss q -> Ln q -> Exp q -> bcast q -> muls q
    def c2_norm_quarter(qq):
        off = qq * 512
        osl = bass.ds(off, 512)
        rb = ps_mm.tile([128, LOCAL], F32, tag="mm", name=f"rl_2_{qq}")
        nc.tensor.matmul(rb[:, 0:512], ones8[:], sqt[2][:, :, osl],
                         start=True, stop=True, perf_mode=DR)
        nc.scalar.activation(lnrow[2][0:1, osl], rb[0:1, 0:512],
                             AF.Ln, bias=0.0, scale=1.0)
        nc.scalar.activation(invrow["p"][0:1, osl], lnrow[2][0:1, osl],
                             AF.Exp, bias=0.0, scale=-0.5)
        bcb = ps_mm.tile([128, LOCAL], F32, tag="mm", name=f"invbl_{qq}")
        bc = bcb[:, 0:512]
        nc.tensor.matmul(bc, ones_r[:], invrow["p"][0:1, osl],
                         start=True, stop=True)
        for k in range(KC):
            nc.vector.tensor_mul(znt8[:, k, bass.ds(CHUNK + off, 512)],
                                 rtk[(2, k)][:, osl], bc)

    pair_counter = [0]

    def emit_pair(c, q):
        pi = pair_counter[0]
        pair_counter[0] += 1
        e8 = sb_e8.tile([128, 2, LOCAL], FP8, tag="e8")
        e8s[pi] = e8
        for par in range(2):
            tt = 2 * q + par
            pm = ps_mm.tile([128, LOCAL], F32, tag="mm")
            for h in range(2):
                nc.tensor.matmul(pm[:, bass.ds(h * 512, 512)],
                                 stationary(c, tt),
                                 znt8[:, :, bass.ds(h * 512, 512)],
                                 start=True, stop=True, perf_mode=DR)
            normed = c in (0, 2)
            eng = ENGINE_MAP[(c, tt)]
            if eng == "act":
                if normed:
                    nc.scalar.activation(e8[:, par, :], pm[:], AF.Exp,
                                         bias=neg2[:], scale=2.0)
                else:
                    nc.scalar.activation(e8[:, par, :], pm[:], AF.Exp,
                                         bias=neg2[:],
                                         scale=inv2c[c][:, bass.ds(tt, 1)])
            else:
                if normed:
                    nc.vector.tensor_scalar(e8[:, par, :].bitcast(U8), pm[:],
                                            SCHR_A, SCHR_B,
                                            ALU.mult, ALU.add)
                else:
                    nc.vector.tensor_scalar(e8[:, par, :].bitcast(U8), pm[:],
                                            invAc[c][:, bass.ds(tt, 1)],
                                            SCHR_B, ALU.mult, ALU.add)
        if pi >= SUM_LAG:
            emit_sum(pi - SUM_LAG, pi == SUM_LAG, False)

    def emit_pos():
        # positive-pair cosines (fp8 TT on GPSIMD + free-dim reduce on
        # DVE); pool ops land after the prep queue, reduces ride the
        # DVE stream
        for k in range(KC):
            nc.gpsimd.tensor_tensor(dmy_pos[k][:], znt8[:, k, 0:LOCAL],
                                    znt8[:, k, bass.ds(CHUNK, LOCAL)],
                                    ALU.mult)
        for k in range(KC):
            nc.vector.reduce_sum(pacc[:, bass.ds(k, 1)], dmy_pos[k][:],
                                 axis=X)

    # ---- emission: c0 pairs first; the c2 quarter chains ride along,
    # then c2 pairs pad the stretch where c1's norm and casts run ----
    for q in (0, 1, 2):
        emit_pair(0, q)
    c2_norm_quarter(0)
    emit_pair(0, 3)
    c2_norm_quarter(1)
    emit_pair(0, 4)
    c2_norm_quarter(2)
    emit_pair(0, 5)
    c2_norm_quarter(3)
    for q in (6, 7):
        emit_pair(0, q)
    # pool prep for the raw chunks (gpsimd order: after the c2 squares)
    squares(1)
    cast8(1, "gp")
    squares(3)
    cast8(3, "gp")
    for q in (0, 1, 2, 3):
        emit_pair(2, q)
    ln_quarter(1, 0)
    emit_pair(2, 4)
    ln_quarter(1, 1)
    emit_pair(2, 5)
    ln_quarter(1, 2)
    emit_pair(2, 6)
    ln_quarter(1, 3)
    emit_pair(2, 7)
    ln_compact(1)
    for q in (0, 1, 2, 3):
        emit_pair(1, q)
    ln_quarter(3, 0)
    emit_pair(1, 4)
    ln_quarter(3, 1)
    emit_pair(1, 5)
    ln_quarter(3, 2)
    emit_pair(1, 6)
    ln_quarter(3, 3)
    emit_pair(1, 7)
    ln_compact(3)
    emit_pos()
    for q in range(8):
        emit_pair(3, q)
    for pj in range(n_pairs - SUM_LAG, n_pairs):
        emit_sum(pj, False, pj == n_pairs - 1)

    # =====================  final reduction  ============================
    for h in range(2):
        nc.scalar.activation(dmy_ln[:], sums[h][:], AF.Ln, bias=neg1[:],
                             scale=1.0, accum_out=fin2[:, bass.ds(h, 1)])
    nc.sync.dma_start(out=out_ap[:, 0:2], in_=fin2[:])
    nc.sync.dma_start(out=out_ap[:, 2:4], in_=pacc[:])


def _pin_act_table(nc):
    # natural_log_exp_and_others (id 6) holds Ln, Exp, Copy: zero reloads.
    nc.scalar.add_instruction(mybir.InstLoadActFuncSet(
        name=nc.get_next_instruction_name(), ins=[], outs=[],
        act_func_set_id=6))


_NC_CACHE = None


def _build_program():
    global _NC_CACHE
    if _NC_CACHE is not None:
        return _NC_CACHE
    nc = bacc.Bacc("TRN2", target_bir_lowering=False, debug=False,
                   num_devices=N_CORES)
    rt = nc.dram_tensor("rt", [KC, 128, N], F32, kind="ExternalInput").ap()
    out = nc.dram_tensor("out", [128, 4], F32, kind="ExternalOutput").ap()
    with tile.TileContext(nc) as tc:
        _pin_act_table(nc)
        _ntxent_kernel(tc, rt, out)
    nc.finalize()
    _NC_CACHE = nc
    return nc


def _prep_inputs(zis: np.ndarray, zjs: np.ndarray) -> list[dict]:
    # Host prep (layout-level only): stack, transpose to [D, N], split
    # the contraction dim, narrow to bf16, and roll columns so each
    # core's local block is at a uniform offset.
    rt_full = np.ascontiguousarray(
        np.concatenate([zjs, zis], axis=0).T.astype(np.float32, copy=False)
    ).reshape(KC, 128, N)

    in_maps = []
    for c in range(N_CORES):
        rolled = np.roll(rt_full, -c * LOCAL, axis=2)
        in_maps.append({"rt": np.ascontiguousarray(rolled)})
    return in_maps


def kernel(zis: np.ndarray, zjs: np.ndarray) -> np.ndarray:
    assert zis.shape == (B, D) and zjs.shape == (B, D)
    nc = _build_program()
    in_maps = _prep_inputs(zis, zjs)
    res = run_bass_kernel_spmd(nc, in_maps, core_ids=list(range(N_CORES)))

    log_sum = 0.0
    pos_sum = 0.0
    for c in range(N_CORES):
        o = res.results[c]["out"]
        log_sum += float(o[0, 0]) + float(o[0, 1])
        pos_sum += float(o[:, 2].sum()) + float(o[:, 3].sum())
    loss = 2.0 + (log_sum - 2.0 * pos_sum) / N
    return np.asarray(loss, dtype=np.float32)


# revision 56
# speedup vs baseline: 1.1075x; 1.0554x over previous
"""NT-Xent loss on 8 Trainium2 NeuronCores (Bass/Tile) — v3.

Math (stacked order rows = [zjs; zis], pair(i) = i +- B):
    cos_ij = (r_i . r_j) * inv_i * inv_j,   inv = 1/|r|
    loss   = 2 + mean_i( ln(rowsum_i - 1) - 2*cos_{i,pair(i)} )
    rowsum_i = sum_j exp(2 cos_ij - 2)   (diagonal contributes exactly 1)

Design (per core, inputs rolled so local rows = cols [0:1024)):
- TRANSPOSED similarity blocks: PSUM block = [128 j-cols, 1024 local-i].
  The j-side norm 1/|r_j| rides the PER-PARTITION scale AP of the exp
  instruction, so only the 2048 local+partner columns are explicitly
  normalized (fp8 znt8); the stationary side stays RAW fp8.
- Main matmul fp8e4 DoubleRow (K=256/instr): stationary = raw fp8
  j-tile, moving = normalized local columns. 2 matmuls per j-tile.
- exp split across THREE pointwise engines (ACT / DVE / GPSIMD):
    ACT: exp8 = fp8(exp((2 inv_j)*S - 2))          [scale AP]
    DVE/GPSIMD: bits = round((23.083 inv_j)*S + 32.517)  [Schraudolph
         e4m3, round-to-nearest uint8 out, C=0.4 calibrated; GPSIMD
         output is bit-identical to DVE]
- Row sums on the PE: ones[128,2,128] fp8-DR matmuls accumulate
  sum_j exp8 into PSUM [128, 512] x2 (broadcast over partitions);
  final = one Ln(bias=-1)+accum per half.
- Norms: squares are written as fp8e4 in k-interleaved layout
  [128, KC, CHUNK], so one fp8-DoubleRow ones-matmul per 512-quarter
  computes the colsum (K=256, 0.5 cyc/col): 4x cheaper than the bf16
  row-layout colsums. c0/c2 keep the row path (Ln + Exp on [1,512]
  rows) to produce inv rows for the normalize broadcasts; c1/c3 skip
  row-Lns entirely: raw ss rows bounce PSUM->DRAM->[128,16] compact
  (DRAM APs transpose freely) and Ln+Exp run on the tiny compact tile.
"""

import numpy as np
from contextlib import ExitStack

import concourse.bass as bass
import concourse.tile as tile
from concourse import bacc, mybir
from concourse.bass_utils import run_bass_kernel_spmd
from concourse._compat import with_exitstack

B = 4096
D = 256
N = 2 * B                 # 8192 similarity rows/cols
N_CORES = 8
LOCAL = N // N_CORES      # 1024 rows per core
CHUNK = 2048
NCHUNK = N // CHUNK       # 4
KC = D // 128             # 2 contraction subtiles of 128
NTILES = N // 128         # 64 j-tiles
TPC = CHUNK // 128        # 16 j-tiles per chunk

F32 = mybir.dt.float32
F32R = mybir.dt.float32r
BF16 = mybir.dt.bfloat16
FP8 = mybir.dt.float8e4
U8 = mybir.dt.uint8
AF = mybir.ActivationFunctionType
ALU = mybir.AluOpType
X = mybir.AxisListType.X
DR = mybir.MatmulPerfMode.DoubleRow

LN2 = float(np.log(2.0))
A8 = 8.0 / LN2                      # e4m3 Schraudolph slope per unit arg
SCHR_B = 56.0 - 2.0 * A8 - 0.4     # bits = (2*A8*inv)*S + SCHR_B
LN_2A8 = float(np.log(2.0 * A8))   # ACT bias producing (2*A8)*inv
SCHR_A = 2.0 * A8                  # const slope for normalized blocks
SUM_LAG = 5                        # sum-matmul software pipelining lag

CORDER = (0, 2, 1, 3)              # local, partner, rest

# per-chunk (ACT, DVE) tile quotas; pairs interleave par0->ACT par1->DVE
ENGINE_QUOTA = {0: 8, 2: 8, 1: 9, 3: 9}   # ACT tiles per chunk


def _engine_map():
    m = {}
    for c in range(4):
        a_left = ENGINE_QUOTA[c]
        d_left = 16 - a_left
        for q in range(8):
            for par in range(2):
                tt = 2 * q + par
                want_act = (par == 0)
                if want_act and a_left > 0 or d_left == 0:
                    m[(c, tt)] = "act"; a_left -= 1
                else:
                    m[(c, tt)] = "dve"; d_left -= 1
    return m


ENGINE_MAP = _engine_map()


@with_exitstack
def _ntxent_kernel(ctx: ExitStack, tc: tile.TileContext, rt_ap, out_ap):
    nc = tc.nc

    sb_rt = ctx.enter_context(tc.tile_pool(name="rt", bufs=2 * NCHUNK))
    sb_big = ctx.enter_context(tc.tile_pool(name="big", bufs=1))
    sb_sq = ctx.enter_context(tc.tile_pool(name="sq", bufs=4))
    sb_e8 = ctx.enter_context(tc.tile_pool(name="e8", bufs=8))
    sb_fin = ctx.enter_context(tc.tile_pool(name="fin", bufs=1))
    dram = ctx.enter_context(tc.tile_pool(name="dram", bufs=1, space="DRAM"))

    # ---- constants ----
    ones_r = sb_fin.tile([1, 128], BF16, tag="ones_r")      # K=1 bcast
    nc.vector.memset(ones_r[:], 1.0)
    ones8 = sb_fin.tile([128, 2, 128], FP8, tag="ones8")    # DR row-sum
    nc.vector.memset(ones8[:], 1.0)
    neg1 = sb_fin.tile([128, 1], F32, tag="neg1")
    nc.vector.memset(neg1[:], -1.0)
    neg2 = sb_fin.tile([128, 1], F32, tag="neg2")
    nc.vector.memset(neg2[:], -2.0)
    bln2 = sb_fin.tile([128, 1], F32, tag="bln2")
    nc.vector.memset(bln2[:], LN2)
    blnA = sb_fin.tile([128, 1], F32, tag="blnA")
    nc.vector.memset(blnA[:], LN_2A8)

    # ---- persistent tiles ----
    rt8 = sb_big.tile([128, KC, N], FP8, tag="rt8")            # raw fp8
    znt8 = sb_fin.tile([128, KC, 2 * CHUNK], FP8, tag="znt8")  # normed c0|c2
    pacc = sb_fin.tile([128, KC], F32, tag="pacc")
    fin2 = sb_fin.tile([128, 2], F32, tag="fin2")

    lnrow = {}      # per chunk: ln(ss) row layout [1, 2048] f32
    lncomp = {}     # c1/c3: ln(ss) compact [128, 16] (DMA-transposed)
    inv2c = {}      # scale 2*inv
    invAc = {}      # scale 23.083*inv
    for c in range(NCHUNK):
        lnrow[c] = sb_fin.tile([1, CHUNK], F32, tag=f"lnrow{c}", name=f"lnrow{c}")
    for c in (1, 3):
        lncomp[c] = sb_fin.tile([128, TPC], F32, tag=f"lncomp{c}", name=f"lncomp{c}")
        inv2c[c] = sb_fin.tile([128, TPC], F32, tag=f"inv2c{c}", name=f"inv2c{c}")
        invAc[c] = sb_fin.tile([128, TPC], F32, tag=f"invAc{c}", name=f"invAc{c}")
    # plain-inv rows (bf16) for chunks 0/2
    invrow = {}
    for nm in ("l", "p"):
        invrow[nm] = sb_fin.tile([1, CHUNK], BF16, tag=f"invrow{nm}", name=f"invrow{nm}")

    dmy_pos = [sb_fin.tile([128, LOCAL], BF16, tag=f"dmy_pos{k}",
                           name=f"dmy_pos{k}") for k in range(KC)]
    dmy_ln = sb_fin.tile([128, 512], BF16, tag="dmy_ln")

    # ---- input DMAs ----
    # c0/c2 arrive as 512-col quarters ordered quarter-major so the
    # per-quarter norm chains can chase the data; c1/c3 as whole chunks.
    rtk = {}
    for c in CORDER:
        for k in range(KC):
            rtk[(c, k)] = sb_rt.tile([128, CHUNK], F32, tag="rt",
                                     name=f"rt_{c}_{k}")
    for c in CORDER:
        if c in (0, 2, 1):
            for h in range(4):
                for k in range(KC):
                    hs = bass.ds(h * 512, 512)
                    nc.sync.dma_start(
                        out=rtk[(c, k)][0:128, hs],
                        in_=rt_ap[k][:, bass.ds(c * CHUNK + h * 512, 512)])
        else:
            for k in range(KC):
                nc.sync.dma_start(out=rtk[(c, k)][:],
                                  in_=rt_ap[k][:, bass.ds(c * CHUNK, CHUNK)])

    # squares, fp8 k-interleaved: sqt[c] = [128, KC, CHUNK]
    sqt = {}
    for c in range(NCHUNK):
        sqt[c] = sb_sq.tile([128, KC, CHUNK], FP8, tag="sq", name=f"sq_{c}")

    def squares(c, engine=None):
        eng = engine or nc.gpsimd
        for k in range(KC):
            eng.tensor_tensor(sqt[c][:, k, :], rtk[(c, k)][:], rtk[(c, k)][:],
                              ALU.mult)

    def cast8(c, eng="dve"):
        for k in range(KC):
            if eng == "dve":
                nc.vector.tensor_copy(rt8[:, k, bass.ds(c * CHUNK, CHUNK)],
                                      rtk[(c, k)][:])
            elif eng == "gp":
                # mult-by-1 tensor_scalar: ~2x faster than gpsimd copy
                nc.gpsimd.tensor_scalar(rt8[:, k, bass.ds(c * CHUNK, CHUNK)],
                                        rtk[(c, k)][:], 1.0, 0.0,
                                        ALU.mult, ALU.add)
            else:
                nc.scalar.activation(rt8[:, k, bass.ds(c * CHUNK, CHUNK)],
                                     rtk[(c, k)][:], AF.Copy,
                                     bias=0.0, scale=1.0)

    def squares_q(c, qq, eng):
        qsl = bass.ds(qq * 512, 512)
        for k in range(KC):
            if eng == "dve":
                nc.vector.tensor_mul(sqt[c][:, k, qsl], rtk[(c, k)][:, qsl],
                                     rtk[(c, k)][:, qsl])
            else:
                nc.gpsimd.tensor_tensor(sqt[c][:, k, qsl],
                                        rtk[(c, k)][:, qsl],
                                        rtk[(c, k)][:, qsl], ALU.mult)

    # bulk emission keeps every square op unblocked in its engine's
    # lookahead window while the chain muls wait on the bcast roundtrip
    for qq in range(4):
        squares_q(0, qq, "dve")
    for qq in range(4):
        squares_q(2, qq, "gp")

    # standalone weight loads: free PE activity to hold the p-state up
    # through windows where the matmul stream has a dependency bubble
    def pe_fill(n):
        for _ in range(n):
            nc.tensor.ldweights(ones8[:], perf_mode=DR)

    # =====================  chunk-0 norm (scoped PSUM)  =================
    # chunk 0: per-quarter chain rowss -> Ln -> Exp -> bcast -> apply
    def c0_quarter(qq, rpool, bpool):
        off = qq * 512
        osl = bass.ds(off, 512)
        rb = rpool.tile([128, 512], F32, tag="rowss", name=f"rq_{qq}")
        nc.tensor.matmul(rb[:], ones8[:], sqt[0][:, :, osl],
                         start=True, stop=True, perf_mode=DR)
        nc.scalar.activation(lnrow[0][0:1, osl], rb[0:1, :],
                             AF.Ln, bias=0.0, scale=1.0)
        nc.scalar.activation(invrow["l"][0:1, osl], lnrow[0][0:1, osl],
                             AF.Exp, bias=0.0, scale=-0.5)
        bcb = bpool.tile([128, 512], F32, tag="invb", name=f"invb_{qq}")
        nc.tensor.matmul(bcb[:], ones_r[:], invrow["l"][0:1, osl],
                         start=True, stop=True)
        for k in range(KC):
            nc.vector.tensor_mul(znt8[:, k, osl],
                                 rtk[(0, k)][:, osl], bcb[:])

    with ExitStack() as norm_ctx:
        ps_row = norm_ctx.enter_context(
            tc.tile_pool(name="psrow", bufs=2, space="PSUM"))
        ps_bc = norm_ctx.enter_context(
            tc.tile_pool(name="psbc", bufs=2, space="PSUM"))
        ps_warm = norm_ctx.enter_context(
            tc.tile_pool(name="pswarm", bufs=1, space="PSUM"))

        # PE p-state warm-up while waiting for the first colsum input.
        warm = ps_warm.tile([128, 128], F32, tag="warm")

        def warm_fill(n):
            for _ in range(n):
                nc.tensor.matmul(warm[:], ones8[:], ones8[:],
                                 start=True, stop=True, perf_mode=DR,
                                 skip_group_check=True)

        warm_fill(14)
        for qq in range(4):
            c0_quarter(qq, ps_row, ps_bc)
            # real-matmul fillers: the OOO window executes these while
            # the chain waits on the ACT Ln/Exp roundtrip, holding the
            # p-state up through the head
            warm_fill(10)

    # =====================  main phase  =================================
    ps_mm = ctx.enter_context(tc.tile_pool(name="psmm", bufs=3, space="PSUM"))
    ps_sum = ctx.enter_context(tc.tile_pool(name="pssum", bufs=1, space="PSUM"))

    sums = [ps_sum.tile([128, 512], F32, tag=f"sum{h}", name=f"sum{h}")
            for h in range(2)]

    n_pairs = NTILES // 2
    pair_list = []
    for c in CORDER:
        for q in range(TPC // 2):
            pair_list.append((c, q))

    # stationary for j-tile t: normalized znt8 slice for chunks 0/2
    # (znt8 = [local | partner]), raw rt8 for chunks 1/3
    def stationary(c, tt):
        if c == 0:
            return znt8[:, :, bass.ds(tt * 128, 128)]
        if c == 2:
            return znt8[:, :, bass.ds(CHUNK + tt * 128, 128)]
        return rt8[:, :, bass.ds((c * TPC + tt) * 128, 128)]

    e8s = {}

    def emit_sum(pj, first, last):
        e8p = e8s.pop(pj)
        for h in range(2):
            nc.tensor.matmul(sums[h][:], ones8[:],
                             e8p[:, :, bass.ds(h * 512, 512)],
                             start=first, stop=last,
                             perf_mode=DR, skip_group_check=True)

    # c1/c3 norm quarter: one fp8-DR colsum matmul + Ln of the
    # (partition-broadcast) PSUM row into the SBUF ln row
    def ln_quarter(c, qq):
        rb = ps_mm.tile([128, LOCAL], F32, tag="mm", name=f"rl_{c}_{qq}")
        sl = bass.ds(qq * 512, 512)
        nc.tensor.matmul(rb[:, 0:512], ones8[:], sqt[c][:, :, sl],
                         start=True, stop=True, perf_mode=DR)
        nc.scalar.activation(lnrow[c][0:1, sl], rb[0:1, 0:512],
                             AF.Ln, bias=0.0, scale=1.0)

    # c1/c3: ln rows -> compact [128, 16] via a DRAM bounce (DRAM APs
    # transpose freely), then the two scale tiles
    def ln_compact(c):
        lnd = dram.tile([1, CHUNK], F32, tag=f"lnd{c}", name=f"lnd{c}")
        nc.sync.dma_start(out=lnd[:, :], in_=lnrow[c][:, :])
        nc.sync.dma_start(
            out=lncomp[c][:],
            in_=lnd[0:1, :].rearrange("o (t p) -> (o p) t", p=128))
        nc.scalar.activation(inv2c[c][:], lncomp[c][:], AF.Exp,
                             bias=bln2[:], scale=-0.5)
        nc.scalar.activation(invAc[c][:], lncomp[c][:], AF.Exp,
                             bias=blnA[:], scale=-0.5)

    # chunk-2 norm, injected into the loop. Quarter-chained exactly like
    # the chunk-0 head chain so partner znt8 becomes usable per-512-slab:
    # sq q -> rowss q -> Ln q -> Exp q -> bcast q -> muls q
    def c2_norm_quarter(qq):
        off = qq * 512
        osl = bass.ds(off, 512)
        rb = ps_mm.tile([128, LOCAL], F32, tag="mm", name=f"rl_2_{qq}")
        nc.tensor.matmul(rb[:, 0:512], ones8[:], sqt[2][:, :, osl],
                         start=True, stop=True, perf_mode=DR)
        nc.scalar.activation(lnrow[2][0:1, osl], rb[0:1, 0:512],
                             AF.Ln, bias=0.0, scale=1.0)
        nc.scalar.activation(invrow["p"][0:1, osl], lnrow[2][0:1, osl],
                             AF.Exp, bias=0.0, scale=-0.5)
        bcb = ps_mm.tile([128, LOCAL], F32, tag="mm", name=f"invbl_{qq}")
        bc = bcb[:, 0:512]
        nc.tensor.matmul(bc, ones_r[:], invrow["p"][0:1, osl],
                         start=True, stop=True)
        for k in range(KC):
            nc.vector.tensor_mul(znt8[:, k, bass.ds(CHUNK + off, 512)],
                                 rtk[(2, k)][:, osl], bc)

    pair_counter = [0]

    def emit_pair(c, q):
        pi = pair_counter[0]
        pair_counter[0] += 1
        e8 = sb_e8.tile([128, 2, LOCAL], FP8, tag="e8")
        e8s[pi] = e8
        for par in range(2):
            tt = 2 * q + par
            pm = ps_mm.tile([128, LOCAL], F32, tag="mm")
            for h in range(2):
                nc.tensor.matmul(pm[:, bass.ds(h * 512, 512)],
                                 stationary(c, tt),
                                 znt8[:, :, bass.ds(h * 512, 512)],
                                 start=True, stop=True, perf_mode=DR)
            normed = c in (0, 2)
            eng = ENGINE_MAP[(c, tt)]
            if eng == "act":
                if normed:
                    nc.scalar.activation(e8[:, par, :], pm[:], AF.Exp,
                                         bias=neg2[:], scale=2.0)
                else:
                    nc.scalar.activation(e8[:, par, :], pm[:], AF.Exp,
                                         bias=neg2[:],
                                         scale=inv2c[c][:, bass.ds(tt, 1)])
            else:
                if normed:
                    nc.vector.tensor_scalar(e8[:, par, :].bitcast(U8), pm[:],
                                            SCHR_A, SCHR_B,
                                            ALU.mult, ALU.add)
                else:
                    nc.vector.tensor_scalar(e8[:, par, :].bitcast(U8), pm[:],
                                            invAc[c][:, bass.ds(tt, 1)],
                                            SCHR_B, ALU.mult, ALU.add)
        if pi >= SUM_LAG:
            emit_sum(pi - SUM_LAG, pi == SUM_LAG, False)

    def emit_pos():
        # positive-pair cosines: fp8 TT on GPSIMD mid-loop; the DVE
        # free-dim reduces run post-loop, overlapping the final Lns
        for k in range(KC):
            nc.gpsimd.tensor_tensor(dmy_pos[k][:], znt8[:, k, 0:LOCAL],
                                    znt8[:, k, bass.ds(CHUNK, LOCAL)],
                                    ALU.mult)

    # ---- emission: c0 pairs first; the c2 quarter chains ride along,
    # then c2 pairs pad the stretch where c1's norm and casts run ----
    for q in (0, 1, 2):
        emit_pair(0, q)
    c2_norm_quarter(0)
    emit_pair(0, 3)
    c2_norm_quarter(1)
    emit_pair(0, 4)
    c2_norm_quarter(2)
    emit_pair(0, 5)
    c2_norm_quarter(3)
    for q in (6, 7):
        emit_pair(0, q)
    # pool prep for the raw chunks (gpsimd order: after the c2 squares).
    # c1 squares in quarters so the spread ln_quarter(1) colsums can
    # chase them as the c1 DMA lands, instead of waiting whole-k ops
    for qq in range(4):
        squares_q(1, qq, "gp")
    cast8(1, "gp")
    squares(3)
    cast8(3, "gp")
    emit_pair(2, 0)
    emit_pair(2, 1)
    ln_quarter(1, 0)
    emit_pair(2, 2)
    ln_quarter(1, 1)
    emit_pair(2, 3)
    ln_quarter(1, 2)
    emit_pair(2, 4)
    ln_quarter(1, 3)
    emit_pair(2, 5)
    ln_compact(1)
    emit_pair(2, 6)
    emit_pair(2, 7)
    ln_quarter(3, 0)
    ln_quarter(3, 1)
    emit_pair(1, 0)
    ln_quarter(3, 2)
    emit_pair(1, 1)
    ln_quarter(3, 3)
    emit_pair(1, 2)
    ln_compact(3)
    emit_pair(1, 3)
    emit_pos()
    for q in (4, 5, 6, 7):
        emit_pair(1, q)
    for q in range(8):
        emit_pair(3, q)
    for pj in range(n_pairs - SUM_LAG, n_pairs):
        emit_sum(pj, False, pj == n_pairs - 1)
    for k in range(KC):
        nc.vector.reduce_sum(pacc[:, bass.ds(k, 1)], dmy_pos[k][:], axis=X)

    # =====================  final reduction  ============================
    for h in range(2):
        nc.scalar.activation(dmy_ln[:], sums[h][:], AF.Ln, bias=neg1[:],
                             scale=1.0, accum_out=fin2[:, bass.ds(h, 1)])
    nc.sync.dma_start(out=out_ap[:, 0:2], in_=fin2[:])
    nc.sync.dma_start(out=out_ap[:, 2:4], in_=pacc[:])


def _pin_act_table(nc):
    # natural_log_exp_and_others (id 6) holds Ln, Exp, Copy: zero reloads.
    nc.scalar.add_instruction(mybir.InstLoadActFuncSet(
        name=nc.get_next_instruction_name(), ins=[], outs=[],
        act_func_set_id=6))


_NC_CACHE = None


def _build_program():
    global _NC_CACHE
    if _NC_CACHE is not None:
        return _NC_CACHE
    nc = bacc.Bacc("TRN2", target_bir_lowering=False, debug=False,
                   num_devices=N_CORES)
    rt = nc.dram_tensor("rt", [KC, 128, N], F32, kind="ExternalInput").ap()
    out = nc.dram_tensor("out", [128, 4], F32, kind="ExternalOutput").ap()
    with tile.TileContext(nc) as tc:
        _pin_act_table(nc)
        _ntxent_kernel(tc, rt, out)
    nc.finalize()
    _NC_CACHE = nc
    return nc


def _prep_inputs(zis: np.ndarray, zjs: np.ndarray) -> list[dict]:
    # Host prep (layout-level only): stack, transpose to [D, N], split
    # the contraction dim, narrow to bf16, and roll columns so each
    # core's local block is at a uniform offset.
    rt_full = np.ascontiguousarray(
        np.concatenate([zjs, zis], axis=0).T.astype(np.float32, copy=False)
    ).reshape(KC, 128, N)

    in_maps = []
    for c in range(N_CORES):
        rolled = np.roll(rt_full, -c * LOCAL, axis=2)
        in_maps.append({"rt": np.ascontiguousarray(rolled)})
    return in_maps


def kernel(zis: np.ndarray, zjs: np.ndarray) -> np.ndarray:
    assert zis.shape == (B, D) and zjs.shape == (B, D)
    nc = _build_program()
    in_maps = _prep_inputs(zis, zjs)
    res = run_bass_kernel_spmd(nc, in_maps, core_ids=list(range(N_CORES)))

    log_sum = 0.0
    pos_sum = 0.0
    for c in range(N_CORES):
        o = res.results[c]["out"]
        log_sum += float(o[0, 0]) + float(o[0, 1])
        pos_sum += float(o[:, 2].sum()) + float(o[:, 3].sum())
    loss = 2.0 + (log_sum - 2.0 * pos_sum) / N
    return np.asarray(loss, dtype=np.float32)


# revision 57
# speedup vs baseline: 1.1273x; 1.0179x over previous
"""NT-Xent loss on 8 Trainium2 NeuronCores (Bass/Tile) — v3.

Math (stacked order rows = [zjs; zis], pair(i) = i +- B):
    cos_ij = (r_i . r_j) * inv_i * inv_j,   inv = 1/|r|
    loss   = 2 + mean_i( ln(rowsum_i - 1) - 2*cos_{i,pair(i)} )
    rowsum_i = sum_j exp(2 cos_ij - 2)   (diagonal contributes exactly 1)

Design (per core, inputs rolled so local rows = cols [0:1024)):
- TRANSPOSED similarity blocks: PSUM block = [128 j-cols, 1024 local-i].
  The j-side norm 1/|r_j| rides the PER-PARTITION scale AP of the exp
  instruction, so only the 2048 local+partner columns are explicitly
  normalized (fp8 znt8); the stationary side stays RAW fp8.
- Main matmul fp8e4 DoubleRow (K=256/instr): stationary = raw fp8
  j-tile, moving = normalized local columns. 2 matmuls per j-tile.
- exp split across THREE pointwise engines (ACT / DVE / GPSIMD):
    ACT: exp8 = fp8(exp((2 inv_j)*S - 2))          [scale AP]
    DVE/GPSIMD: bits = round((23.083 inv_j)*S + 32.517)  [Schraudolph
         e4m3, round-to-nearest uint8 out, C=0.4 calibrated; GPSIMD
         output is bit-identical to DVE]
- Row sums on the PE: ones[128,2,128] fp8-DR matmuls accumulate
  sum_j exp8 into PSUM [128, 512] x2 (broadcast over partitions);
  final = one Ln(bias=-1)+accum per half.
- Norms: squares are written as fp8e4 in k-interleaved layout
  [128, KC, CHUNK], so one fp8-DoubleRow ones-matmul per 512-quarter
  computes the colsum (K=256, 0.5 cyc/col): 4x cheaper than the bf16
  row-layout colsums. c0/c2 keep the row path (Ln + Exp on [1,512]
  rows) to produce inv rows for the normalize broadcasts; c1/c3 skip
  row-Lns entirely: raw ss rows bounce PSUM->DRAM->[128,16] compact
  (DRAM APs transpose freely) and Ln+Exp run on the tiny compact tile.
"""

import numpy as np
from contextlib import ExitStack

import concourse.bass as bass
import concourse.tile as tile
from concourse import bacc, mybir
from concourse.bass_utils import run_bass_kernel_spmd
from concourse._compat import with_exitstack

B = 4096
D = 256
N = 2 * B                 # 8192 similarity rows/cols
N_CORES = 8
LOCAL = N // N_CORES      # 1024 rows per core
CHUNK = 2048
NCHUNK = N // CHUNK       # 4
KC = D // 128             # 2 contraction subtiles of 128
NTILES = N // 128         # 64 j-tiles
TPC = CHUNK // 128        # 16 j-tiles per chunk

F32 = mybir.dt.float32
F32R = mybir.dt.float32r
BF16 = mybir.dt.bfloat16
FP8 = mybir.dt.float8e4
U8 = mybir.dt.uint8
AF = mybir.ActivationFunctionType
ALU = mybir.AluOpType
X = mybir.AxisListType.X
DR = mybir.MatmulPerfMode.DoubleRow

LN2 = float(np.log(2.0))
A8 = 8.0 / LN2                      # e4m3 Schraudolph slope per unit arg
SCHR_B = 56.0 - 2.0 * A8 - 0.4     # bits = (2*A8*inv)*S + SCHR_B
LN_2A8 = float(np.log(2.0 * A8))   # ACT bias producing (2*A8)*inv
SCHR_A = 2.0 * A8                  # const slope for normalized blocks
SUM_LAG = 5                        # sum-matmul software pipelining lag

CORDER = (0, 2, 1, 3)              # local, partner, rest

# per-chunk (ACT, DVE) tile quotas; pairs interleave par0->ACT par1->DVE
ENGINE_QUOTA = {0: 8, 2: 8, 1: 9, 3: 9}   # ACT tiles per chunk


def _engine_map():
    m = {}
    for c in range(4):
        a_left = ENGINE_QUOTA[c]
        d_left = 16 - a_left
        for q in range(8):
            for par in range(2):
                tt = 2 * q + par
                want_act = (par == 0)
                if want_act and a_left > 0 or d_left == 0:
                    m[(c, tt)] = "act"; a_left -= 1
                else:
                    m[(c, tt)] = "dve"; d_left -= 1
    return m


ENGINE_MAP = _engine_map()


@with_exitstack
def _ntxent_kernel(ctx: ExitStack, tc: tile.TileContext, rt_ap, out_ap):
    nc = tc.nc

    sb_rt = ctx.enter_context(tc.tile_pool(name="rt", bufs=2 * NCHUNK))
    sb_big = ctx.enter_context(tc.tile_pool(name="big", bufs=1))
    sb_sq = ctx.enter_context(tc.tile_pool(name="sq", bufs=4))
    sb_e8 = ctx.enter_context(tc.tile_pool(name="e8", bufs=8))
    sb_fin = ctx.enter_context(tc.tile_pool(name="fin", bufs=1))
    dram = ctx.enter_context(tc.tile_pool(name="dram", bufs=1, space="DRAM"))

    # ---- constants ----
    ones_r = sb_fin.tile([1, 128], BF16, tag="ones_r")      # K=1 bcast
    nc.vector.memset(ones_r[:], 1.0)
    ones8 = sb_fin.tile([128, 2, 128], FP8, tag="ones8")    # DR row-sum
    nc.vector.memset(ones8[:], 1.0)
    neg1 = sb_fin.tile([128, 1], F32, tag="neg1")
    nc.vector.memset(neg1[:], -1.0)
    neg2 = sb_fin.tile([128, 1], F32, tag="neg2")
    nc.vector.memset(neg2[:], -2.0)
    bln2 = sb_fin.tile([128, 1], F32, tag="bln2")
    nc.vector.memset(bln2[:], LN2)
    blnA = sb_fin.tile([128, 1], F32, tag="blnA")
    nc.vector.memset(blnA[:], LN_2A8)

    # ---- persistent tiles ----
    rt8 = sb_big.tile([128, KC, N], FP8, tag="rt8")            # raw fp8
    znt8 = sb_fin.tile([128, KC, 2 * CHUNK], FP8, tag="znt8")  # normed c0|c2
    pacc = sb_fin.tile([128, KC], F32, tag="pacc")
    fin2 = sb_fin.tile([128, 2], F32, tag="fin2")

    lnrow = {}      # per chunk: ln(ss) row layout [1, 2048] f32
    lncomp = {}     # c1/c3: ln(ss) compact [128, 16] (DMA-transposed)
    inv2c = {}      # scale 2*inv
    invAc = {}      # scale 23.083*inv
    for c in range(NCHUNK):
        lnrow[c] = sb_fin.tile([1, CHUNK], F32, tag=f"lnrow{c}", name=f"lnrow{c}")
    for c in (1, 3):
        lncomp[c] = sb_fin.tile([128, TPC], F32, tag=f"lncomp{c}", name=f"lncomp{c}")
        inv2c[c] = sb_fin.tile([128, TPC], F32, tag=f"inv2c{c}", name=f"inv2c{c}")
        invAc[c] = sb_fin.tile([128, TPC], F32, tag=f"invAc{c}", name=f"invAc{c}")
    # plain-inv rows (bf16) for chunks 0/2
    invrow = {}
    for nm in ("l", "p"):
        invrow[nm] = sb_fin.tile([1, CHUNK], BF16, tag=f"invrow{nm}", name=f"invrow{nm}")

    dmy_pos = [sb_fin.tile([128, LOCAL], BF16, tag=f"dmy_pos{k}",
                           name=f"dmy_pos{k}") for k in range(KC)]
    dmy_ln = sb_fin.tile([128, 512], BF16, tag="dmy_ln")

    # ---- input DMAs ----
    # c0/c2 arrive as 512-col quarters ordered quarter-major so the
    # per-quarter norm chains can chase the data; c1/c3 as whole chunks.
    rtk = {}
    for c in CORDER:
        for k in range(KC):
            rtk[(c, k)] = sb_rt.tile([128, CHUNK], F32, tag="rt",
                                     name=f"rt_{c}_{k}")
    for c in CORDER:
        if c in (0, 2, 1):
            for h in range(4):
                for k in range(KC):
                    hs = bass.ds(h * 512, 512)
                    nc.sync.dma_start(
                        out=rtk[(c, k)][0:128, hs],
                        in_=rt_ap[k][:, bass.ds(c * CHUNK + h * 512, 512)])
        else:
            for k in range(KC):
                nc.sync.dma_start(out=rtk[(c, k)][:],
                                  in_=rt_ap[k][:, bass.ds(c * CHUNK, CHUNK)])

    # squares, fp8 k-interleaved: sqt[c] = [128, KC, CHUNK]
    sqt = {}
    for c in range(NCHUNK):
        sqt[c] = sb_sq.tile([128, KC, CHUNK], FP8, tag="sq", name=f"sq_{c}")

    def squares(c, engine=None):
        eng = engine or nc.gpsimd
        for k in range(KC):
            eng.tensor_tensor(sqt[c][:, k, :], rtk[(c, k)][:], rtk[(c, k)][:],
                              ALU.mult)

    def cast8(c, eng="dve"):
        for k in range(KC):
            if eng == "dve":
                nc.vector.tensor_copy(rt8[:, k, bass.ds(c * CHUNK, CHUNK)],
                                      rtk[(c, k)][:])
            elif eng == "gp":
                # mult-by-1 tensor_scalar: ~2x faster than gpsimd copy
                nc.gpsimd.tensor_scalar(rt8[:, k, bass.ds(c * CHUNK, CHUNK)],
                                        rtk[(c, k)][:], 1.0, 0.0,
                                        ALU.mult, ALU.add)
            else:
                nc.scalar.activation(rt8[:, k, bass.ds(c * CHUNK, CHUNK)],
                                     rtk[(c, k)][:], AF.Copy,
                                     bias=0.0, scale=1.0)

    def squares_q(c, qq, eng):
        qsl = bass.ds(qq * 512, 512)
        for k in range(KC):
            if eng == "dve":
                nc.vector.tensor_mul(sqt[c][:, k, qsl], rtk[(c, k)][:, qsl],
                                     rtk[(c, k)][:, qsl])
            else:
                nc.gpsimd.tensor_tensor(sqt[c][:, k, qsl],
                                        rtk[(c, k)][:, qsl],
                                        rtk[(c, k)][:, qsl], ALU.mult)

    # bulk emission keeps every square op unblocked in its engine's
    # lookahead window while the chain muls wait on the bcast roundtrip
    for qq in range(4):
        squares_q(0, qq, "dve")
    for qq in range(4):
        squares_q(2, qq, "gp")

    # standalone weight loads: free PE activity to hold the p-state up
    # through windows where the matmul stream has a dependency bubble
    def pe_fill(n):
        for _ in range(n):
            nc.tensor.ldweights(ones8[:], perf_mode=DR)

    # =====================  chunk-0 norm (scoped PSUM)  =================
    # chunk 0: per-quarter chain rowss -> Ln -> Exp -> bcast -> apply
    def c0_quarter(qq, rpool, bpool):
        off = qq * 512
        osl = bass.ds(off, 512)
        rb = rpool.tile([128, 512], F32, tag="rowss", name=f"rq_{qq}")
        nc.tensor.matmul(rb[:], ones8[:], sqt[0][:, :, osl],
                         start=True, stop=True, perf_mode=DR)
        nc.scalar.activation(lnrow[0][0:1, osl], rb[0:1, :],
                             AF.Ln, bias=0.0, scale=1.0)
        nc.scalar.activation(invrow["l"][0:1, osl], lnrow[0][0:1, osl],
                             AF.Exp, bias=0.0, scale=-0.5)
        bcb = bpool.tile([128, 512], F32, tag="invb", name=f"invb_{qq}")
        nc.tensor.matmul(bcb[:], ones_r[:], invrow["l"][0:1, osl],
                         start=True, stop=True)
        for k in range(KC):
            nc.vector.tensor_mul(znt8[:, k, osl],
                                 rtk[(0, k)][:, osl], bcb[:])

    with ExitStack() as norm_ctx:
        ps_row = norm_ctx.enter_context(
            tc.tile_pool(name="psrow", bufs=2, space="PSUM"))
        ps_bc = norm_ctx.enter_context(
            tc.tile_pool(name="psbc", bufs=2, space="PSUM"))
        ps_warm = norm_ctx.enter_context(
            tc.tile_pool(name="pswarm", bufs=1, space="PSUM"))

        # PE p-state warm-up while waiting for the first colsum input.
        warm = ps_warm.tile([128, 128], F32, tag="warm")

        def warm_fill(n):
            for _ in range(n):
                nc.tensor.matmul(warm[:], ones8[:], ones8[:],
                                 start=True, stop=True, perf_mode=DR,
                                 skip_group_check=True)

        warm_fill(14)
        for qq in range(4):
            c0_quarter(qq, ps_row, ps_bc)
            # real-matmul fillers: the OOO window executes these while
            # the chain waits on the ACT Ln/Exp roundtrip, holding the
            # p-state up through the head; after the last quarter real
            # pair work exists, so leftover fillers would only delay it
            warm_fill(10 if qq < 3 else 4)

    # =====================  main phase  =================================
    ps_mm = ctx.enter_context(tc.tile_pool(name="psmm", bufs=3, space="PSUM"))
    ps_sum = ctx.enter_context(tc.tile_pool(name="pssum", bufs=1, space="PSUM"))

    sums = [ps_sum.tile([128, 512], F32, tag=f"sum{h}", name=f"sum{h}")
            for h in range(2)]

    n_pairs = NTILES // 2
    pair_list = []
    for c in CORDER:
        for q in range(TPC // 2):
            pair_list.append((c, q))

    # stationary for j-tile t: normalized znt8 slice for chunks 0/2
    # (znt8 = [local | partner]), raw rt8 for chunks 1/3
    def stationary(c, tt):
        if c == 0:
            return znt8[:, :, bass.ds(tt * 128, 128)]
        if c == 2:
            return znt8[:, :, bass.ds(CHUNK + tt * 128, 128)]
        return rt8[:, :, bass.ds((c * TPC + tt) * 128, 128)]

    e8s = {}

    def emit_sum(pj, first, last):
        e8p = e8s.pop(pj)
        for h in range(2):
            nc.tensor.matmul(sums[h][:], ones8[:],
                             e8p[:, :, bass.ds(h * 512, 512)],
                             start=first, stop=last,
                             perf_mode=DR, skip_group_check=True)

    # c1/c3 norm quarter: one fp8-DR colsum matmul + Ln of the
    # (partition-broadcast) PSUM row into the SBUF ln row
    def ln_quarter(c, qq):
        rb = ps_mm.tile([128, LOCAL], F32, tag="mm", name=f"rl_{c}_{qq}")
        sl = bass.ds(qq * 512, 512)
        nc.tensor.matmul(rb[:, 0:512], ones8[:], sqt[c][:, :, sl],
                         start=True, stop=True, perf_mode=DR)
        nc.scalar.activation(lnrow[c][0:1, sl], rb[0:1, 0:512],
                             AF.Ln, bias=0.0, scale=1.0)

    # c1/c3: ln rows -> compact [128, 16] via a DRAM bounce (DRAM APs
    # transpose freely), then the two scale tiles
    def ln_compact(c):
        lnd = dram.tile([1, CHUNK], F32, tag=f"lnd{c}", name=f"lnd{c}")
        nc.sync.dma_start(out=lnd[:, :], in_=lnrow[c][:, :])
        nc.sync.dma_start(
            out=lncomp[c][:],
            in_=lnd[0:1, :].rearrange("o (t p) -> (o p) t", p=128))
        nc.scalar.activation(inv2c[c][:], lncomp[c][:], AF.Exp,
                             bias=bln2[:], scale=-0.5)
        nc.scalar.activation(invAc[c][:], lncomp[c][:], AF.Exp,
                             bias=blnA[:], scale=-0.5)

    # chunk-2 norm, injected into the loop. Quarter-chained exactly like
    # the chunk-0 head chain so partner znt8 becomes usable per-512-slab:
    # sq q -> rowss q -> Ln q -> Exp q -> bcast q -> muls q
    def c2_norm_quarter(qq):
        off = qq * 512
        osl = bass.ds(off, 512)
        rb = ps_mm.tile([128, LOCAL], F32, tag="mm", name=f"rl_2_{qq}")
        nc.tensor.matmul(rb[:, 0:512], ones8[:], sqt[2][:, :, osl],
                         start=True, stop=True, perf_mode=DR)
        nc.scalar.activation(lnrow[2][0:1, osl], rb[0:1, 0:512],
                             AF.Ln, bias=0.0, scale=1.0)
        nc.scalar.activation(invrow["p"][0:1, osl], lnrow[2][0:1, osl],
                             AF.Exp, bias=0.0, scale=-0.5)
        bcb = ps_mm.tile([128, LOCAL], F32, tag="mm", name=f"invbl_{qq}")
        bc = bcb[:, 0:512]
        nc.tensor.matmul(bc, ones_r[:], invrow["p"][0:1, osl],
                         start=True, stop=True)
        for k in range(KC):
            nc.vector.tensor_mul(znt8[:, k, bass.ds(CHUNK + off, 512)],
                                 rtk[(2, k)][:, osl], bc)

    pair_counter = [0]

    def emit_pair(c, q):
        pi = pair_counter[0]
        pair_counter[0] += 1
        e8 = sb_e8.tile([128, 2, LOCAL], FP8, tag="e8")
        e8s[pi] = e8
        for par in range(2):
            tt = 2 * q + par
            pm = ps_mm.tile([128, LOCAL], F32, tag="mm")
            for h in range(2):
                nc.tensor.matmul(pm[:, bass.ds(h * 512, 512)],
                                 stationary(c, tt),
                                 znt8[:, :, bass.ds(h * 512, 512)],
                                 start=True, stop=True, perf_mode=DR)
            normed = c in (0, 2)
            eng = ENGINE_MAP[(c, tt)]
            if eng == "act":
                if normed:
                    nc.scalar.activation(e8[:, par, :], pm[:], AF.Exp,
                                         bias=neg2[:], scale=2.0)
                else:
                    nc.scalar.activation(e8[:, par, :], pm[:], AF.Exp,
                                         bias=neg2[:],
                                         scale=inv2c[c][:, bass.ds(tt, 1)])
            else:
                if normed:
                    nc.vector.tensor_scalar(e8[:, par, :].bitcast(U8), pm[:],
                                            SCHR_A, SCHR_B,
                                            ALU.mult, ALU.add)
                else:
                    nc.vector.tensor_scalar(e8[:, par, :].bitcast(U8), pm[:],
                                            invAc[c][:, bass.ds(tt, 1)],
                                            SCHR_B, ALU.mult, ALU.add)
        if pi >= SUM_LAG:
            emit_sum(pi - SUM_LAG, pi == SUM_LAG, False)

    def emit_pos():
        # positive-pair cosines: fp8 TT on GPSIMD mid-loop; the DVE
        # free-dim reduces run post-loop, overlapping the final Lns
        for k in range(KC):
            nc.gpsimd.tensor_tensor(dmy_pos[k][:], znt8[:, k, 0:LOCAL],
                                    znt8[:, k, bass.ds(CHUNK, LOCAL)],
                                    ALU.mult)

    # ---- emission: c0 pairs first; the c2 quarter chains ride along,
    # then c2 pairs pad the stretch where c1's norm and casts run ----
    for q in (0, 1, 2):
        emit_pair(0, q)
    c2_norm_quarter(0)
    emit_pair(0, 3)
    c2_norm_quarter(1)
    emit_pair(0, 4)
    c2_norm_quarter(2)
    emit_pair(0, 5)
    c2_norm_quarter(3)
    for q in (6, 7):
        emit_pair(0, q)
    # pool prep for the raw chunks (gpsimd order: after the c2 squares).
    # c1 squares in quarters so the spread ln_quarter(1) colsums can
    # chase them as the c1 DMA lands, instead of waiting whole-k ops
    for qq in range(4):
        squares_q(1, qq, "gp")
    cast8(1, "gp")
    squares(3)
    cast8(3, "gp")
    emit_pair(2, 0)
    emit_pair(2, 1)
    ln_quarter(1, 0)
    emit_pair(2, 2)
    ln_quarter(1, 1)
    emit_pair(2, 3)
    ln_quarter(1, 2)
    emit_pair(2, 4)
    ln_quarter(1, 3)
    emit_pair(2, 5)
    ln_compact(1)
    emit_pair(2, 6)
    emit_pair(2, 7)
    ln_quarter(3, 0)
    ln_quarter(3, 1)
    emit_pair(1, 0)
    ln_quarter(3, 2)
    emit_pair(1, 1)
    ln_quarter(3, 3)
    emit_pair(1, 2)
    ln_compact(3)
    emit_pair(1, 3)
    emit_pos()
    for q in (4, 5, 6, 7):
        emit_pair(1, q)
    for q in range(8):
        emit_pair(3, q)
    for pj in range(n_pairs - SUM_LAG, n_pairs):
        emit_sum(pj, False, pj == n_pairs - 1)
    for k in range(KC):
        nc.vector.reduce_sum(pacc[:, bass.ds(k, 1)], dmy_pos[k][:], axis=X)

    # =====================  final reduction  ============================
    for h in range(2):
        nc.scalar.activation(dmy_ln[:], sums[h][:], AF.Ln, bias=neg1[:],
                             scale=1.0, accum_out=fin2[:, bass.ds(h, 1)])
    nc.sync.dma_start(out=out_ap[:, 0:2], in_=fin2[:])
    nc.sync.dma_start(out=out_ap[:, 2:4], in_=pacc[:])


def _pin_act_table(nc):
    # natural_log_exp_and_others (id 6) holds Ln, Exp, Copy: zero reloads.
    nc.scalar.add_instruction(mybir.InstLoadActFuncSet(
        name=nc.get_next_instruction_name(), ins=[], outs=[],
        act_func_set_id=6))


_NC_CACHE = None


def _build_program():
    global _NC_CACHE
    if _NC_CACHE is not None:
        return _NC_CACHE
    nc = bacc.Bacc("TRN2", target_bir_lowering=False, debug=False,
                   num_devices=N_CORES)
    rt = nc.dram_tensor("rt", [KC, 128, N], F32, kind="ExternalInput").ap()
    out = nc.dram_tensor("out", [128, 4], F32, kind="ExternalOutput").ap()
    with tile.TileContext(nc) as tc:
        _pin_act_table(nc)
        _ntxent_kernel(tc, rt, out)
    nc.finalize()
    _NC_CACHE = nc
    return nc


def _prep_inputs(zis: np.ndarray, zjs: np.ndarray) -> list[dict]:
    # Host prep (layout-level only): stack, transpose to [D, N], split
    # the contraction dim, narrow to bf16, and roll columns so each
    # core's local block is at a uniform offset.
    rt_full = np.ascontiguousarray(
        np.concatenate([zjs, zis], axis=0).T.astype(np.float32, copy=False)
    ).reshape(KC, 128, N)

    in_maps = []
    for c in range(N_CORES):
        rolled = np.roll(rt_full, -c * LOCAL, axis=2)
        in_maps.append({"rt": np.ascontiguousarray(rolled)})
    return in_maps


def kernel(zis: np.ndarray, zjs: np.ndarray) -> np.ndarray:
    assert zis.shape == (B, D) and zjs.shape == (B, D)
    nc = _build_program()
    in_maps = _prep_inputs(zis, zjs)
    res = run_bass_kernel_spmd(nc, in_maps, core_ids=list(range(N_CORES)))

    log_sum = 0.0
    pos_sum = 0.0
    for c in range(N_CORES):
        o = res.results[c]["out"]
        log_sum += float(o[0, 0]) + float(o[0, 1])
        pos_sum += float(o[:, 2].sum()) + float(o[:, 3].sum())
    loss = 2.0 + (log_sum - 2.0 * pos_sum) / N
    return np.asarray(loss, dtype=np.float32)
